# revision 20
# baseline (speedup 1.0000x reference)
"""Sinenet kernel for 8 Trainium2 NeuronCores (Bass/Tile, SPMD data-parallel).

Math per row n (of S*B*M = 2560 rows), with f = exp(nlf*std + mean):
  deg[k,t] = (k+1)*2pi*f*(t*T_WAV - tau)
  X[n, 0:32]  = sum_t sin(deg[k,t]) * wav[n,t]
  X[n, 32:64] = sum_t cos(deg[k,t]) * wav[n,t]
  out[n] = relu(X @ W1 + [nlf,tau,vuv] @ W2 + b1 + b2)

Device pipeline (layout B: t on partitions, (row, kk) on free dim):
  - PE matmul builds the angle in 1/8192-turn fixed point:
      psi8[u, j] = u*A2p8[j] + B2c8[j] + MAGIC
    The fp32 PSUM write rounds to the nearest integer count (magic-number
    rounding; MAGIC = 1.5*2^23 rides its own contraction row, summed last).
  - DVE tensor_scalar extracts the fractional angle counts:
      bits = psi8_bits AND 0x1FFF   (int32 counts i in [0, 8192))
  - ScalarE Sin reads the int32 counts (converted on read):
      trig = Sin(i * 2pi/8192 - pi)  == sin(deg) / cos(deg) per column half.
  - PE matmuls contract over t per row (trig stationary, wav column moving),
    accumulating X columns in PSUM across the 5 t-chunks.
  - Final PE matmuls: X@W1 + a@W2 + bias (rank-1), ScalarE relu, DMA out.

Host does sharding plus tiny coefficient staging (A2/B2 rows, wav transpose);
all O(N*K*T) math runs on device.
"""

import numpy as np

S, B, Mm, T = 4, 16, 40, 640
K, D = 32, 512
T_WAV = 1.0 / 24000.0
LOG_F_MEAN = 5.04418
LOG_F_STD = 0.358402
NCORES = 8
ROWS = S * B * Mm          # 2560
RPC = ROWS // NCORES       # 320 rows per core
NG = RPC // 8              # 40 groups of 8 rows
NCH = T // 128             # 5 t-chunks
GPT = 20                   # groups per RHS tensor (2 blocks of 10)
MAGIC = 12582912.0         # 1.5 * 2^23
FIXS = 8192.0              # fixed-point counts per turn

_CACHE = {}


def _build_nc():
    import concourse.bacc as bacc
    import concourse.tile as tile
    import concourse.mybir as mybir

    A = mybir.AluOpType
    F32 = mybir.dt.float32
    I32 = mybir.dt.int32
    AF = mybir.ActivationFunctionType

    nc = bacc.Bacc(None, target_bir_lowering=False)

    rhs_d = [nc.dram_tensor(f"RHS{i}", [64, 512 * NCH], F32, kind="ExternalInput")
             for i in range(2)]
    wavt = nc.dram_tensor("WAVT", [T, RPC], F32, kind="ExternalInput")
    lhst = nc.dram_tensor("LHST", [64, 128 * 10], F32, kind="ExternalInput")
    w1 = nc.dram_tensor("W1", [2 * K, D], F32, kind="ExternalInput")
    w2 = nc.dram_tensor("W2", [3, D], F32, kind="ExternalInput")
    b1 = nc.dram_tensor("B1", [D], F32, kind="ExternalInput")
    b2 = nc.dram_tensor("B2", [D], F32, kind="ExternalInput")
    at = nc.dram_tensor("AT", [3, RPC], F32, kind="ExternalInput")
    out = nc.dram_tensor("OUT", [RPC, D], F32, kind="ExternalOutput")
    outx = nc.dram_tensor("OUTX", [2 * K, RPC], F32, kind="ExternalOutput")

    with tile.TileContext(nc) as tc:
        with (
            tc.tile_pool(name="const", bufs=1) as cpool,
            tc.tile_pool(name="work", bufs=6) as work,
            tc.tile_pool(name="trigp", bufs=6) as trigp,
            tc.tile_pool(name="ps_psi", bufs=5, space="PSUM") as ps_psi,
            tc.tile_pool(name="ps_x", bufs=1, space="PSUM") as ps_x,
            tc.tile_pool(name="ps_y", bufs=1, space="PSUM") as ps_y,
        ):
            t_rhs = []
            for i in range(2):
                r = cpool.tile([64, 512 * NCH], F32, tag=f"rhs{i}")
                nc.sync.dma_start(r[:], rhs_d[i][:])
                t_rhs.append(r)
            t_wav = []
            for c in range(NCH):
                w = cpool.tile([128, RPC], F32, tag=f"wavt{c}")
                nc.sync.dma_start(w[:], wavt[128 * c:128 * (c + 1), :])
                t_wav.append(w)
            t_lhs = cpool.tile([64, 128 * 10], F32)
            nc.sync.dma_start(t_lhs[:], lhst[:])
            t_w1 = cpool.tile([2 * K, D], F32)
            nc.sync.dma_start(t_w1[:], w1[:])
            t_w2 = cpool.tile([3, D], F32)
            nc.sync.dma_start(t_w2[:], w2[:])
            t_b1 = cpool.tile([1, D], F32)
            nc.sync.dma_start(t_b1[:], b1.ap().unsqueeze(0))
            t_b2 = cpool.tile([1, D], F32)
            nc.sync.dma_start(t_b2[:], b2.ap().unsqueeze(0))
            t_bsum = cpool.tile([1, D], F32)
            nc.vector.tensor_tensor(t_bsum[:], t_b1[:], t_b2[:], A.add)
            t_at = cpool.tile([3, RPC], F32)
            nc.sync.dma_start(t_at[:], at[:])
            ones1 = cpool.tile([1, RPC], F32)
            nc.vector.memset(ones1[:], 1.0)
            cbias = cpool.tile([128, 1], F32)
            nc.vector.memset(cbias[:], float(-(np.pi - 1e-6)))
            cscale = cpool.tile([128, 1], F32)
            nc.vector.memset(cscale[:], float(2.0 * np.pi / FIXS))
            xt = cpool.tile([2 * K, RPC], F32)

            for g in range(NG):
                ti, within = g // GPT, g % GPT
                qb, p = (within // 10) * 32, within % 10
                psx = ps_x.tile([2 * K, 8], F32)
                nc.vector.memset(psx[:], 0.0)
                for c in range(NCH):
                    ps = ps_psi.tile([128, 512], F32)
                    nc.tensor.matmul(
                        ps[:], t_lhs[qb:qb + 32, 128 * p:128 * (p + 1)],
                        t_rhs[ti][qb:qb + 32, 512 * c:512 * (c + 1)],
                        start=True, stop=True,
                    )
                    bits = work.tile([128, 512], I32, tag="bits")
                    nc.vector.tensor_scalar(
                        bits[:], ps[:].bitcast(I32), 8191, 0,
                        A.bitwise_and, A.bitwise_or,
                    )
                    trig = trigp.tile([128, 512], F32, tag="trig")
                    nc.scalar.activation(
                        trig[:], bits[:], AF.Sin,
                        bias=cbias[:], scale=cscale[:],
                    )
                    for r8 in range(8):
                        row = 8 * g + r8
                        nc.tensor.matmul(
                            psx[:, r8:r8 + 1],
                            trig[:, 64 * r8:64 * r8 + 64],
                            t_wav[c][:, row:row + 1],
                            start=False, stop=False,
                            skip_group_check=True,
                        )
                nc.vector.tensor_copy(xt[:, 8 * g:8 * g + 8], psx[:])

            nc.sync.dma_start(outx[:], xt[:])
            for lo, n in ((0, 128), (128, 128), (256, 64)):
                psy = ps_y.tile([128, D], F32)
                nc.tensor.matmul(psy[:n, :], xt[:, lo:lo + n], t_w1[:],
                                 start=True, stop=False, skip_group_check=True)
                nc.tensor.matmul(psy[:n, :], t_at[:, lo:lo + n], t_w2[:],
                                 start=False, stop=False, skip_group_check=True)
                nc.tensor.matmul(psy[:n, :], ones1[:, lo:lo + n], t_bsum[:],
                                 start=False, stop=True, skip_group_check=True)
                h = work.tile([128, D], F32, tag="h")
                nc.scalar.activation(h[:n, :], psy[:n, :], AF.Relu)
                nc.sync.dma_start(out[lo:lo + n, :], h[:n, :])

    nc.finalize()
    return nc


def _host_stage(wav, nlf, tau, vuv):
    """Per-core input maps. wav/nlf/tau/vuv are [ROWS,...] row-major."""
    kk = np.arange(64)
    kmul = (kk % 32 + 1).astype(np.float64)          # harmonic number per j-col
    cos_half = (kk >= 32).astype(np.float64) * 0.25  # cos phase shift in turns

    f = np.exp(nlf.astype(np.float64) * LOG_F_STD + LOG_F_MEAN)        # [ROWS]
    A2 = f[:, None] * T_WAV * kmul[None, :]                            # [ROWS,64]
    A2p = A2 - np.rint(A2)
    # +0.5 turn folded in so that sin(2pi*frac - pi) == sin(deg)
    B2 = 0.5 + cos_half[None, :] \
        - f[:, None] * tau.astype(np.float64)[:, None] * kmul[None, :]

    # 10 masked lhsT variants: variant p selects in-block rows 3p..3p+2 as
    # {iota*u, 1, 1} and zeroes the rest. MAGIC row (3p+2) last in the block.
    lhst = np.zeros((64, 128 * 10), np.float32)
    for q in (0, 32):
        for p in range(10):
            lhst[q + 3 * p + 0, 128 * p:128 * (p + 1)] = np.arange(128)
            lhst[q + 3 * p + 1, 128 * p:128 * (p + 1)] = 1.0
            lhst[q + 3 * p + 2, 128 * p:128 * (p + 1)] = 1.0

    maps = []
    for core in range(NCORES):
        sl = slice(core * RPC, (core + 1) * RPC)
        A2c, B2c = A2p[sl], B2[sl]
        a2g = (FIXS * A2c).reshape(NG, 512)          # j = 64*r8 + kk per group
        b2g = np.empty((NG, 512 * NCH))
        for c in range(NCH):
            x = B2c + 128.0 * c * A2c
            b2g[:, 512 * c:512 * (c + 1)] = (FIXS * (x - np.rint(x))).reshape(NG, 512)
        rhs = [np.zeros((64, 512 * NCH), np.float32) for _ in range(2)]
        for g in range(NG):
            ti, within = g // GPT, g % GPT
            base = (within // 10) * 32 + 3 * (within % 10)
            rhs[ti][base + 0, :] = np.tile(a2g[g], NCH)
            rhs[ti][base + 1, :] = b2g[g]
            rhs[ti][base + 2, :] = MAGIC
        maps.append({
            "RHS0": rhs[0],
            "RHS1": rhs[1],
            "WAVT": np.ascontiguousarray(wav[sl].T.astype(np.float32)),
            "LHST": lhst,
            "AT": np.ascontiguousarray(
                np.stack([nlf[sl], tau[sl], vuv[sl]]).astype(np.float32)),
        })
    return maps


def kernel(wav_SBMT, nlf_SBM, tau_SBM, vuv_SBM, W1, b1, W2, b2):
    from concourse.bass_utils import run_bass_kernel_spmd

    if "nc" not in _CACHE:
        _CACHE["nc"] = _build_nc()
    nc = _CACHE["nc"]

    wav = np.ascontiguousarray(wav_SBMT, dtype=np.float32).reshape(ROWS, T)
    nlf = np.asarray(nlf_SBM, dtype=np.float32).reshape(ROWS)
    tau = np.asarray(tau_SBM, dtype=np.float32).reshape(ROWS)
    vuv = np.asarray(vuv_SBM, dtype=np.float32).reshape(ROWS)

    maps = _host_stage(wav, nlf, tau, vuv)
    shared = {
        "W1": np.ascontiguousarray(W1, dtype=np.float32),
        "W2": np.ascontiguousarray(W2, dtype=np.float32),
        "B1": np.ascontiguousarray(b1, dtype=np.float32),
        "B2": np.ascontiguousarray(b2, dtype=np.float32),
    }
    for m in maps:
        m.update(shared)

    res = run_bass_kernel_spmd(nc, maps, core_ids=list(range(NCORES)))
    _CACHE["last_results"] = res
    full = np.concatenate([r["OUT"] for r in res.results], axis=0)
    return full.reshape(S, B, Mm, D).astype(np.float32)


# revision 21
# speedup vs baseline: 1.0086x; 1.0086x over previous
"""Sinenet kernel for 8 Trainium2 NeuronCores (Bass/Tile, SPMD data-parallel).

Math per row n (of S*B*M = 2560 rows), with f = exp(nlf*std + mean):
  deg[k,t] = (k+1)*2pi*f*(t*T_WAV - tau)
  X[n, 0:32]  = sum_t sin(deg[k,t]) * wav[n,t]
  X[n, 32:64] = sum_t cos(deg[k,t]) * wav[n,t]
  out[n] = relu(X @ W1 + [nlf,tau,vuv] @ W2 + b1 + b2)

Device pipeline (layout B: t on partitions, (row, kk) on free dim):
  - PE matmul builds the angle in 1/8192-turn fixed point:
      psi8[u, j] = u*A2p8[j] + B2c8[j] + MAGIC
    The fp32 PSUM write rounds to the nearest integer count (magic-number
    rounding; MAGIC = 1.5*2^23 rides its own contraction row, summed last).
  - DVE tensor_scalar extracts the fractional angle counts:
      bits = psi8_bits AND 0x1FFF   (int32 counts i in [0, 8192))
  - ScalarE Sin reads the int32 counts (converted on read):
      trig = Sin(i * 2pi/8192 - pi)  == sin(deg) / cos(deg) per column half.
  - PE matmuls contract over t per row (trig stationary, wav column moving),
    accumulating X columns in PSUM across the 5 t-chunks.
  - Final PE matmuls: X@W1 + a@W2 + bias (rank-1), ScalarE relu, DMA out.

Host does sharding plus tiny coefficient staging (A2/B2 rows, wav transpose);
all O(N*K*T) math runs on device.
"""

import numpy as np

S, B, Mm, T = 4, 16, 40, 640
K, D = 32, 512
T_WAV = 1.0 / 24000.0
LOG_F_MEAN = 5.04418
LOG_F_STD = 0.358402
NCORES = 8
ROWS = S * B * Mm          # 2560
RPC = ROWS // NCORES       # 320 rows per core
NG = RPC // 8              # 40 groups of 8 rows
NCH = T // 128             # 5 t-chunks
GPT = 20                   # groups per RHS tensor (2 blocks of 10)
MAGIC = 12582912.0         # 1.5 * 2^23
FIXS = 8192.0              # fixed-point counts per turn

_CACHE = {}


def _build_nc():
    import concourse.bacc as bacc
    import concourse.tile as tile
    import concourse.mybir as mybir

    A = mybir.AluOpType
    F32 = mybir.dt.float32
    I32 = mybir.dt.int32
    AF = mybir.ActivationFunctionType

    nc = bacc.Bacc(None, target_bir_lowering=False)

    rhs_d = [nc.dram_tensor(f"RHS{i}", [64, 512 * NCH], F32, kind="ExternalInput")
             for i in range(2)]
    wavt = nc.dram_tensor("WAVT", [T, RPC], F32, kind="ExternalInput")
    lhst = nc.dram_tensor("LHST", [64, 128 * 10], F32, kind="ExternalInput")
    w1 = nc.dram_tensor("W1", [2 * K, D], F32, kind="ExternalInput")
    w2 = nc.dram_tensor("W2", [3, D], F32, kind="ExternalInput")
    b1 = nc.dram_tensor("B1", [D], F32, kind="ExternalInput")
    b2 = nc.dram_tensor("B2", [D], F32, kind="ExternalInput")
    at = nc.dram_tensor("AT", [3, RPC], F32, kind="ExternalInput")
    out = nc.dram_tensor("OUT", [RPC, D], F32, kind="ExternalOutput")
    outx = nc.dram_tensor("OUTX", [2 * K, RPC], F32, kind="ExternalOutput")

    with tile.TileContext(nc) as tc:
        with (
            tc.tile_pool(name="const", bufs=1) as cpool,
            tc.tile_pool(name="work", bufs=6) as work,
            tc.tile_pool(name="trigp", bufs=6) as trigp,
            tc.tile_pool(name="ps_psi", bufs=5, space="PSUM") as ps_psi,
            tc.tile_pool(name="ps_x", bufs=1, space="PSUM") as ps_x,
            tc.tile_pool(name="ps_y", bufs=2, space="PSUM") as ps_y,
        ):
            t_rhs = []
            for i in range(2):
                r = cpool.tile([64, 512 * NCH], F32, tag=f"rhs{i}")
                nc.sync.dma_start(r[:], rhs_d[i][:])
                t_rhs.append(r)
            t_wav = []
            for c in range(NCH):
                w = cpool.tile([128, RPC], F32, tag=f"wavt{c}")
                nc.sync.dma_start(w[:], wavt[128 * c:128 * (c + 1), :])
                t_wav.append(w)
            t_lhs = cpool.tile([64, 128 * 10], F32)
            nc.sync.dma_start(t_lhs[:], lhst[:])
            t_w1 = cpool.tile([2 * K, D], F32)
            nc.sync.dma_start(t_w1[:], w1[:])
            t_w2 = cpool.tile([3, D], F32)
            nc.sync.dma_start(t_w2[:], w2[:])
            t_b1 = cpool.tile([1, D], F32)
            nc.sync.dma_start(t_b1[:], b1.ap().unsqueeze(0))
            t_b2 = cpool.tile([1, D], F32)
            nc.sync.dma_start(t_b2[:], b2.ap().unsqueeze(0))
            t_bsum = cpool.tile([1, D], F32)
            nc.vector.tensor_tensor(t_bsum[:], t_b1[:], t_b2[:], A.add)
            t_at = cpool.tile([3, RPC], F32)
            nc.sync.dma_start(t_at[:], at[:])
            ones1 = cpool.tile([1, RPC], F32)
            nc.vector.memset(ones1[:], 1.0)
            cbias = cpool.tile([128, 1], F32)
            nc.vector.memset(cbias[:], float(-(np.pi - 1e-6)))
            cscale = cpool.tile([128, 1], F32)
            nc.vector.memset(cscale[:], float(2.0 * np.pi / FIXS))
            xt = cpool.tile([2 * K, RPC], F32)

            for g in range(NG):
                ti, within = g // GPT, g % GPT
                qb, p = (within // 10) * 32, within % 10
                psx = ps_x.tile([2 * K, 8], F32)
                nc.vector.memset(psx[:], 0.0)
                for c in range(NCH):
                    ps = ps_psi.tile([128, 512], F32)
                    nc.tensor.matmul(
                        ps[:], t_lhs[qb:qb + 32, 128 * p:128 * (p + 1)],
                        t_rhs[ti][qb:qb + 32, 512 * c:512 * (c + 1)],
                        start=True, stop=True,
                    )
                    bits = work.tile([128, 512], I32, tag="bits")
                    nc.vector.tensor_scalar(
                        bits[:], ps[:].bitcast(I32), 8191, 0,
                        A.bitwise_and, A.bitwise_or,
                    )
                    trig = trigp.tile([128, 512], F32, tag="trig")
                    nc.scalar.activation(
                        trig[:], bits[:], AF.Sin,
                        bias=cbias[:], scale=cscale[:],
                    )
                    for r8 in range(8):
                        row = 8 * g + r8
                        nc.tensor.matmul(
                            psx[:, r8:r8 + 1],
                            trig[:, 64 * r8:64 * r8 + 64],
                            t_wav[c][:, row:row + 1],
                            start=False, stop=False,
                            skip_group_check=True,
                        )
                nc.vector.tensor_copy(xt[:, 8 * g:8 * g + 8], psx[:])

            nc.sync.dma_start(outx[:], xt[:])
            for lo, n in ((0, 128), (128, 128), (256, 64)):
                psy = ps_y.tile([128, D], F32)
                nc.tensor.matmul(psy[:n, :], xt[:, lo:lo + n], t_w1[:],
                                 start=True, stop=False, skip_group_check=True)
                nc.tensor.matmul(psy[:n, :], t_at[:, lo:lo + n], t_w2[:],
                                 start=False, stop=False, skip_group_check=True)
                nc.tensor.matmul(psy[:n, :], ones1[:, lo:lo + n], t_bsum[:],
                                 start=False, stop=True, skip_group_check=True)
                h = work.tile([128, D], F32, tag="h")
                nc.scalar.activation(h[:n, :], psy[:n, :], AF.Relu)
                nc.sync.dma_start(out[lo:lo + n, :], h[:n, :])

    nc.finalize()
    return nc


def _host_stage(wav, nlf, tau, vuv):
    """Per-core input maps. wav/nlf/tau/vuv are [ROWS,...] row-major."""
    kk = np.arange(64)
    kmul = (kk % 32 + 1).astype(np.float64)          # harmonic number per j-col
    cos_half = (kk >= 32).astype(np.float64) * 0.25  # cos phase shift in turns

    f = np.exp(nlf.astype(np.float64) * LOG_F_STD + LOG_F_MEAN)        # [ROWS]
    A2 = f[:, None] * T_WAV * kmul[None, :]                            # [ROWS,64]
    A2p = A2 - np.rint(A2)
    # +0.5 turn folded in so that sin(2pi*frac - pi) == sin(deg)
    B2 = 0.5 + cos_half[None, :] \
        - f[:, None] * tau.astype(np.float64)[:, None] * kmul[None, :]

    # 10 masked lhsT variants: variant p selects in-block rows 3p..3p+2 as
    # {iota*u, 1, 1} and zeroes the rest. MAGIC row (3p+2) last in the block.
    lhst = np.zeros((64, 128 * 10), np.float32)
    for q in (0, 32):
        for p in range(10):
            lhst[q + 3 * p + 0, 128 * p:128 * (p + 1)] = np.arange(128)
            lhst[q + 3 * p + 1, 128 * p:128 * (p + 1)] = 1.0
            lhst[q + 3 * p + 2, 128 * p:128 * (p + 1)] = 1.0

    maps = []
    for core in range(NCORES):
        sl = slice(core * RPC, (core + 1) * RPC)
        A2c, B2c = A2p[sl], B2[sl]
        a2g = (FIXS * A2c).reshape(NG, 512)          # j = 64*r8 + kk per group
        b2g = np.empty((NG, 512 * NCH))
        for c in range(NCH):
            x = B2c + 128.0 * c * A2c
            b2g[:, 512 * c:512 * (c + 1)] = (FIXS * (x - np.rint(x))).reshape(NG, 512)
        rhs = [np.zeros((64, 512 * NCH), np.float32) for _ in range(2)]
        for g in range(NG):
            ti, within = g // GPT, g % GPT
            base = (within // 10) * 32 + 3 * (within % 10)
            rhs[ti][base + 0, :] = np.tile(a2g[g], NCH)
            rhs[ti][base + 1, :] = b2g[g]
            rhs[ti][base + 2, :] = MAGIC
        maps.append({
            "RHS0": rhs[0],
            "RHS1": rhs[1],
            "WAVT": np.ascontiguousarray(wav[sl].T.astype(np.float32)),
            "LHST": lhst,
            "AT": np.ascontiguousarray(
                np.stack([nlf[sl], tau[sl], vuv[sl]]).astype(np.float32)),
        })
    return maps


def kernel(wav_SBMT, nlf_SBM, tau_SBM, vuv_SBM, W1, b1, W2, b2):
    from concourse.bass_utils import run_bass_kernel_spmd

    if "nc" not in _CACHE:
        _CACHE["nc"] = _build_nc()
    nc = _CACHE["nc"]

    wav = np.ascontiguousarray(wav_SBMT, dtype=np.float32).reshape(ROWS, T)
    nlf = np.asarray(nlf_SBM, dtype=np.float32).reshape(ROWS)
    tau = np.asarray(tau_SBM, dtype=np.float32).reshape(ROWS)
    vuv = np.asarray(vuv_SBM, dtype=np.float32).reshape(ROWS)

    maps = _host_stage(wav, nlf, tau, vuv)
    shared = {
        "W1": np.ascontiguousarray(W1, dtype=np.float32),
        "W2": np.ascontiguousarray(W2, dtype=np.float32),
        "B1": np.ascontiguousarray(b1, dtype=np.float32),
        "B2": np.ascontiguousarray(b2, dtype=np.float32),
    }
    for m in maps:
        m.update(shared)

    res = run_bass_kernel_spmd(nc, maps, core_ids=list(range(NCORES)))
    _CACHE["last_results"] = res
    full = np.concatenate([r["OUT"] for r in res.results], axis=0)
    return full.reshape(S, B, Mm, D).astype(np.float32)
